# revision 1
# baseline (speedup 1.0000x reference)
"""Trainium2 Bass kernel for DHGNNRawConv-style GNN message passing.

Math (from the reference):
    h = x @ weight                                   # (N, 256)
    s-branch: region_s = h[edge_neighs]              # (N, 16, 256)
      conved_s[n,c] = sum_t region_s[n,t,c] * Ws[c,t] + bs[c]
      mult_s = softmax over j of conved_s.reshape(n,16,16)
      alpha_s[n,t] = sum_i wK1_s[i] * mult_s[n,i,t]
      x_s[n,:] = sum_t alpha_s[n,t] * region_s[n,t,:] + bK1_s
    k-branch: analogous with 8 neighbors, grouped conv (64 groups of 4 chans)
    attention: softmax over an axis of SIZE 1 -> identically 1.0, so
      out = x_s + x_k + bias        (attention MLP weights are dead)

Distribution: data-parallel over nodes across 8 cores. Each core
computes the full projected-feature table h (replicated matmul; cheap)
into its local DRAM in bf16, then row-gathers its shard's neighbor
regions with indirect DMA and does the per-node conv/softmax/pool math
on DVE/ACT.
"""

import numpy as np

# ---- hardcoded problem geometry ----
N = 50000
D_IN = 128
D_OUT = 256
KS = 16
KK = 8
SLOTS = KS + KK  # 24

NCORES = 8
NP_TOTAL = 50176          # 128 * 392 (padded node count)
PER_CORE = NP_TOTAL // NCORES   # 6272
TILES = PER_CORE // 128         # 49
CHUNKS = NP_TOTAL // 128        # 392 (phase-1 matmul chunks)
SLAB = 1024                     # phase-1 x-slab width (nodes)
NSLABS = NP_TOTAL // SLAB       # 49


# gather implementation: "wrapped16" | "rowmajor" | "flat" | "per_slot"
GATHER_MODE = "per_slot"


def _build_program():
    import concourse.bacc as bacc
    import concourse.tile as tile
    from concourse import mybir
    from concourse.bass import IndirectOffsetOnAxis

    bf16 = mybir.dt.bfloat16
    f32 = mybir.dt.float32
    i32 = mybir.dt.int32
    AF = mybir.ActivationFunctionType
    ALU = mybir.AluOpType
    AX = mybir.AxisListType

    nc = bacc.Bacc("TRN2", target_bir_lowering=False, debug=False,
                   num_devices=NCORES)

    xt_d = nc.dram_tensor("xt", [128, NP_TOTAL], bf16, kind="ExternalInput").ap()
    w_d = nc.dram_tensor("wmat", [128, D_OUT], bf16, kind="ExternalInput").ap()
    widx_d = nc.dram_tensor("widx", [128, TILES * SLOTS], i32,
                            kind="ExternalInput").ap()
    wsexp_d = nc.dram_tensor("wsexp", [128, SLOTS * D_OUT], bf16,
                             kind="ExternalInput").ap()
    wk1r_d = nc.dram_tensor("wk1r", [128, SLOTS], f32, kind="ExternalInput").ap()
    cb_d = nc.dram_tensor("cb", [128, D_OUT + 64], f32, kind="ExternalInput").ap()
    fb_d = nc.dram_tensor("fb", [128, D_OUT], f32, kind="ExternalInput").ap()
    out_d = nc.dram_tensor("out", [PER_CORE, D_OUT], f32,
                           kind="ExternalOutput").ap()

    with tile.TileContext(nc) as tc:
        with (
            tc.tile_pool(name="persist", bufs=1) as persist,
            tc.tile_pool(name="dram", bufs=1, space="DRAM") as dpool,
        ):
            h = dpool.tile([NP_TOTAL, D_OUT], bf16)

            w_sb = persist.tile([128, D_OUT], bf16)
            nc.sync.dma_start(w_sb[:], w_d)
            wsexp_sb = persist.tile([128, SLOTS, D_OUT], bf16)
            nc.sync.dma_start(wsexp_sb[:], wsexp_d.rearrange(
                "p (s c) -> p s c", s=SLOTS))
            wk1r_sb = persist.tile([128, SLOTS], f32)
            nc.sync.dma_start(wk1r_sb[:], wk1r_d)
            cb_sb = persist.tile([128, D_OUT + 64], f32)
            nc.sync.dma_start(cb_sb[:], cb_d)
            fb_sb = persist.tile([128, D_OUT], f32)
            nc.sync.dma_start(fb_sb[:], fb_d)
            idx_sb = persist.tile([128, TILES * SLOTS], i32)
            nc.sync.dma_start(idx_sb[:], widx_d)

            # ---------- phase 1: h = x @ W (full, replicated) ----------
            with (
                tc.tile_pool(name="xsl", bufs=3) as xsl_p,
                tc.tile_pool(name="hsb", bufs=3) as hsb_p,
                tc.tile_pool(name="ps1", bufs=8, space="PSUM") as psum_p,
            ):
                for s in range(NSLABS):
                    xs = xsl_p.tile([128, SLAB], bf16, tag="xs")
                    nc.sync.dma_start(xs[:], xt_d[:, s * SLAB:(s + 1) * SLAB])
                    hs = hsb_p.tile([128, SLAB // 128, D_OUT], bf16, tag="hs")
                    for j in range(SLAB // 128):
                        pt = psum_p.tile([128, D_OUT], f32, tag="pt")
                        nc.tensor.matmul(pt[:], lhsT=xs[:, j * 128:(j + 1) * 128],
                                         rhs=w_sb[:], start=True, stop=True)
                        if j % 2 == 0:
                            nc.vector.tensor_copy(hs[:, j, :], pt[:])
                        else:
                            nc.scalar.activation(hs[:, j, :], pt[:], AF.Copy)
                    nc.sync.dma_start(
                        h[s * SLAB:(s + 1) * SLAB, :].rearrange(
                            "(j p) c -> p j c", p=128),
                        hs[:])

            # ---------- phase 2: gather + conv/softmax/pool ----------
            with (
                tc.tile_pool(name="reg", bufs=3) as reg_p,
                tc.tile_pool(name="work", bufs=2) as work,
            ):
                for t in range(TILES):
                    region = reg_p.tile([128, SLOTS, D_OUT], bf16, tag="region")
                    if GATHER_MODE == "per_slot":
                        # production-proven pattern: one index per partition
                        for s in range(SLOTS):
                            nc.gpsimd.indirect_dma_start(
                                out=region[:, s, :], out_offset=None, in_=h[:, :],
                                in_offset=IndirectOffsetOnAxis(
                                    ap=idx_sb[:, t * SLOTS + s:t * SLOTS + s + 1],
                                    axis=0))
                    else:
                        idxs = idx_sb[:, t * SLOTS:(t + 1) * SLOTS]
                        nc.gpsimd.indirect_dma_start(
                            out=region[:], out_offset=None, in_=h[:, :],
                            in_offset=IndirectOffsetOnAxis(ap=idxs, axis=0))

                    # --- s-branch conved + bias ---
                    scal = work.tile([128, KS, D_OUT], bf16, tag="scal")
                    nc.vector.tensor_mul(scal[:], region[:, 0:KS, :],
                                         wsexp_sb[:, 0:KS, :])
                    t8 = work.tile([128, 8, D_OUT], bf16, tag="t8")
                    nc.vector.tensor_add(t8[:], scal[:, 0:8, :], scal[:, 8:16, :])
                    t4 = work.tile([128, 4, D_OUT], bf16, tag="t4")
                    nc.vector.tensor_add(t4[:], t8[:, 0:4, :], t8[:, 4:8, :])
                    t2 = work.tile([128, 2, D_OUT], bf16, tag="t2")
                    nc.vector.tensor_add(t2[:], t4[:, 0:2, :], t4[:, 2:4, :])
                    t1 = work.tile([128, D_OUT], bf16, tag="t1")
                    nc.vector.tensor_add(t1[:], t2[:, 0, :], t2[:, 1, :])
                    cs = work.tile([128, D_OUT], f32, tag="cs")
                    nc.vector.tensor_add(cs[:], t1[:], cb_sb[:, 0:D_OUT])

                    # --- s softmax -> beta_s ---
                    es = work.tile([128, KS, KS], bf16, tag="es")
                    nc.scalar.activation(es.rearrange("p i j -> p (i j)"),
                                         cs[:], AF.Exp)
                    sume = work.tile([128, KS], f32, tag="sume")
                    nc.vector.tensor_reduce(sume[:], es[:], axis=AX.X, op=ALU.add)
                    rec = work.tile([128, KS], f32, tag="rec")
                    nc.vector.reciprocal(rec[:], sume[:])
                    r2 = work.tile([128, KS], f32, tag="r2")
                    nc.vector.tensor_mul(r2[:], rec[:], wk1r_sb[:, 0:KS])
                    ps_ = work.tile([128, KS, KS], bf16, tag="ps_")
                    nc.vector.tensor_mul(ps_[:], es[:],
                                         r2.to_broadcast([128, KS, KS]))
                    beta = work.tile([128, SLOTS], f32, tag="beta")
                    nc.vector.tensor_reduce(beta[:, 0:KS],
                                            ps_.rearrange("p i j -> p j i"),
                                            axis=AX.X, op=ALU.add)

                    # --- k-branch conved (grouped: 64 out chans x 4 in) ---
                    sck = work.tile([128, KK, D_OUT], bf16, tag="sck")
                    nc.vector.tensor_mul(sck[:], region[:, KS:SLOTS, :],
                                         wsexp_sb[:, KS:SLOTS, :])
                    k4 = work.tile([128, 4, D_OUT], bf16, tag="k4")
                    nc.vector.tensor_add(k4[:], sck[:, 0:4, :], sck[:, 4:8, :])
                    k2 = work.tile([128, 2, D_OUT], bf16, tag="k2")
                    nc.vector.tensor_add(k2[:], k4[:, 0:2, :], k4[:, 2:4, :])
                    k1 = work.tile([128, D_OUT], bf16, tag="k1")
                    nc.vector.tensor_add(k1[:], k2[:, 0, :], k2[:, 1, :])
                    ck = work.tile([128, 64], f32, tag="ck")
                    nc.vector.tensor_reduce(ck[:],
                                            k1.rearrange("p (o i) -> p o i", i=4),
                                            axis=AX.X, op=ALU.add)
                    ckb = work.tile([128, 64], f32, tag="ckb")
                    nc.vector.tensor_add(ckb[:], ck[:], cb_sb[:, D_OUT:D_OUT + 64])

                    # --- k softmax -> beta_k ---
                    ek = work.tile([128, KK, KK], bf16, tag="ek")
                    nc.scalar.activation(ek.rearrange("p i j -> p (i j)"),
                                         ckb[:], AF.Exp)
                    sumk = work.tile([128, KK], f32, tag="sumk")
                    nc.vector.tensor_reduce(sumk[:], ek[:], axis=AX.X, op=ALU.add)
                    reck = work.tile([128, KK], f32, tag="reck")
                    nc.vector.reciprocal(reck[:], sumk[:])
                    r2k = work.tile([128, KK], f32, tag="r2k")
                    nc.vector.tensor_mul(r2k[:], reck[:], wk1r_sb[:, KS:SLOTS])
                    pk_ = work.tile([128, KK, KK], bf16, tag="pk_")
                    nc.vector.tensor_mul(pk_[:], ek[:],
                                         r2k.to_broadcast([128, KK, KK]))
                    nc.vector.tensor_reduce(beta[:, KS:SLOTS],
                                            pk_.rearrange("p i j -> p j i"),
                                            axis=AX.X, op=ALU.add)

                    # --- pooled: sum_s beta[n,s] * region[n,s,:] (+ final bias) ---
                    betab = work.tile([128, SLOTS], bf16, tag="betab")
                    nc.vector.tensor_copy(betab[:], beta[:])
                    pp = work.tile([128, SLOTS, D_OUT], bf16, tag="pp")
                    nc.vector.tensor_mul(pp[:], region[:],
                                         betab.to_broadcast([128, SLOTS, D_OUT]))
                    q12 = work.tile([128, 12, D_OUT], bf16, tag="q12")
                    nc.vector.tensor_add(q12[:], pp[:, 0:12, :], pp[:, 12:24, :])
                    q6 = work.tile([128, 6, D_OUT], bf16, tag="q6")
                    nc.vector.tensor_add(q6[:], q12[:, 0:6, :], q12[:, 6:12, :])
                    q3 = work.tile([128, 3, D_OUT], bf16, tag="q3")
                    nc.vector.tensor_add(q3[:], q6[:, 0:3, :], q6[:, 3:6, :])
                    qa = work.tile([128, D_OUT], bf16, tag="qa")
                    nc.vector.tensor_add(qa[:], q3[:, 0, :], q3[:, 1, :])
                    qb = work.tile([128, D_OUT], bf16, tag="qb")
                    nc.vector.tensor_add(qb[:], qa[:], q3[:, 2, :])
                    outs = work.tile([128, D_OUT], f32, tag="outs")
                    nc.vector.tensor_add(outs[:], qb[:], fb_sb[:])
                    nc.sync.dma_start(out_d[t * 128:(t + 1) * 128, :], outs[:])

    nc.finalize()
    return nc


def _prep_inputs(inputs):
    import ml_dtypes
    bf16 = ml_dtypes.bfloat16

    x = np.asarray(inputs["x"], dtype=np.float32)
    edge = np.asarray(inputs["edge_neighs_index"], dtype=np.int32)
    knn = np.asarray(inputs["knn_neighs_index"], dtype=np.int32)
    W = np.asarray(inputs["weight"], dtype=np.float32)
    bias = np.asarray(inputs["bias"], dtype=np.float32)
    ws = np.asarray(inputs["convKK_s_w"], dtype=np.float32)     # (256,1,16)
    wsb = np.asarray(inputs["convKK_s_b"], dtype=np.float32)    # (256,)
    ws1 = np.asarray(inputs["convK1_s_w"], dtype=np.float32)    # (1,16,1)
    ws1b = np.asarray(inputs["convK1_s_b"], dtype=np.float32)   # (1,)
    wk = np.asarray(inputs["convKK_k_w"], dtype=np.float32)     # (64,4,8)
    wkb = np.asarray(inputs["convKK_k_b"], dtype=np.float32)    # (64,)
    wk1 = np.asarray(inputs["convK1_k_w"], dtype=np.float32)    # (1,8,1)
    wk1b = np.asarray(inputs["convK1_k_b"], dtype=np.float32)   # (1,)

    xp = np.zeros((NP_TOTAL, D_IN), np.float32)
    xp[:N] = x
    xT = np.ascontiguousarray(xp.T).astype(bf16)                 # (128, 50176)
    Wb = W.astype(bf16)                                          # (128, 256)

    merged = np.zeros((NP_TOTAL, SLOTS), np.int32)
    merged[:N, :KS] = edge
    merged[:N, KS:] = knn

    # WsE[t, c] = ws[c, 0, t];  WkE[t, o*4+i] = wk[o, i, t]
    WsE = ws[:, 0, :].T                                          # (16, 256)
    WkE = wk.transpose(2, 0, 1).reshape(KK, 256)                 # (8, 256)
    wsexp = np.concatenate([WsE.reshape(-1), WkE.reshape(-1)])
    wsexp_t = np.ascontiguousarray(
        np.broadcast_to(wsexp, (128, SLOTS * D_OUT))).astype(bf16)

    wk1r = np.ascontiguousarray(np.broadcast_to(
        np.concatenate([ws1[0, :, 0], wk1[0, :, 0]]), (128, SLOTS))
    ).astype(np.float32)
    cb = np.ascontiguousarray(np.broadcast_to(
        np.concatenate([wsb, wkb]), (128, D_OUT + 64))).astype(np.float32)
    fb = np.ascontiguousarray(np.broadcast_to(
        bias + ws1b[0] + wk1b[0], (128, D_OUT))).astype(np.float32)

    in_maps = []
    for c in range(NCORES):
        widx_c = np.ascontiguousarray(
            merged[c * PER_CORE:(c + 1) * PER_CORE]
            .reshape(TILES, 128, SLOTS).transpose(1, 0, 2)
            .reshape(128, TILES * SLOTS))
        in_maps.append({
            "xt": xT, "wmat": Wb, "widx": widx_c, "wsexp": wsexp_t,
            "wk1r": wk1r, "cb": cb, "fb": fb,
        })
    return in_maps


_CACHED_NC = None


def run(inputs, trace=False):
    """Build (cached), run on 8 cores, return (output, BassKernelResults)."""
    global _CACHED_NC
    from concourse.bass_utils import run_bass_kernel_spmd

    if _CACHED_NC is None:
        _CACHED_NC = _build_program()
    nc = _CACHED_NC

    in_maps = _prep_inputs(inputs)
    res = run_bass_kernel_spmd(nc, in_maps, core_ids=list(range(NCORES)),
                               trace=trace)
    shards = [np.asarray(res.results[c]["out"], dtype=np.float32)
              for c in range(NCORES)]
    full = np.concatenate(shards, axis=0)[:N]
    return full, res


def kernel(**inputs) -> np.ndarray:
    out, _ = run(inputs, trace=False)
    return out



# revision 19
# speedup vs baseline: 1.2847x; 1.2847x over previous
"""Trainium2 Bass kernel for DHGNNRawConv-style GNN message passing.

Math (from the reference):
    h = x @ weight                                   # (N, 256)
    s-branch: region_s = h[edge_neighs]              # (N, 16, 256)
      conved_s[n,c] = sum_t region_s[n,t,c] * Ws[c,t] + bs[c]
      mult_s = softmax over j of conved_s.reshape(n,16,16)
      alpha_s[n,t] = sum_i wK1_s[i] * mult_s[n,i,t]
      x_s[n,:] = sum_t alpha_s[n,t] * region_s[n,t,:] + bK1_s
    k-branch: analogous with 8 neighbors, grouped conv (64 groups of 4 chans)
    attention: softmax over an axis of SIZE 1 -> identically 1.0, so
      out = x_s + x_k + bias        (attention MLP weights are dead)

Distribution: data-parallel over nodes across 8 cores. Each core
computes the full projected-feature table h (replicated matmul; cheap)
into its local DRAM in bf16, then row-gathers its shard's neighbor
regions with one batched indirect DMA per tile.

Engine assignment (v2): the three per-node weighted sums run on the
Tensor engine as PSUM-accumulated matmuls -- the slot sums of the
depthwise convs use an identity stationary operand, and the final
pooled sum uses per-slot diagonal matrices diag(beta[:, j]) built with
4x-mode tensor_scalar ops.  exp(conv bias) is premultiplied into the
softmax numerator so no bias add is needed before the exp, and the
final output bias is added with one extra matmul against a
partition-replicated bias row.  DVE keeps only the elementwise
region*W multiply, the softmax reductions, and the diag builds.
"""

import os
import numpy as np

# HW-risk feature flags (sim passes all; bisecting HW divergence)
BATCHED_GATHER = os.environ.get("KBG", "0") == "1"
POOL_MUL = os.environ.get("KPM", "0") == "1"
ACT_DIAG = os.environ.get("KAD", "0") == "1"

# ---- hardcoded problem geometry ----
N = 50000
D_IN = 128
D_OUT = 256
KS = 16
KK = 8
SLOTS = KS + KK  # 24

NCORES = 8
NP_TOTAL = 50176          # 128 * 392 (padded node count)
PER_CORE = NP_TOTAL // NCORES   # 6272
TILES = PER_CORE // 128         # 49
SLAB = 1024                     # phase-1 x-slab width (nodes)
NSLABS = NP_TOTAL // SLAB       # 49


def _build_program(num_devices=NCORES):
    import concourse.bacc as bacc
    import concourse.tile as tile
    from concourse import mybir
    from concourse.bass import IndirectOffsetOnAxis

    bf16 = mybir.dt.bfloat16
    f32 = mybir.dt.float32
    i32 = mybir.dt.int32
    AF = mybir.ActivationFunctionType
    ALU = mybir.AluOpType
    AX = mybir.AxisListType

    nc = bacc.Bacc("TRN2", target_bir_lowering=False, debug=False,
                   num_devices=num_devices)

    xt_d = nc.dram_tensor("xt", [128, NP_TOTAL], bf16, kind="ExternalInput").ap()
    w_d = nc.dram_tensor("wmat", [128, D_OUT], bf16, kind="ExternalInput").ap()
    widx_d = nc.dram_tensor("widx", [128, TILES * SLOTS], i32,
                            kind="ExternalInput").ap()
    wsexp_d = nc.dram_tensor("wsexp", [128, SLOTS * D_OUT], bf16,
                             kind="ExternalInput").ap()
    wk1r_d = nc.dram_tensor("wk1r", [128, SLOTS], f32, kind="ExternalInput").ap()
    ecb_d = nc.dram_tensor("ecb", [128, D_OUT + 64], bf16,
                           kind="ExternalInput").ap()
    fbb_d = nc.dram_tensor("fbb", [128, D_OUT], bf16, kind="ExternalInput").ap()
    ident_d = nc.dram_tensor("ident", [128, 128], bf16,
                             kind="ExternalInput").ap()
    out_d = nc.dram_tensor("out", [PER_CORE, D_OUT], bf16,
                           kind="ExternalOutput").ap()

    with tile.TileContext(nc) as tc:
        with (
            tc.tile_pool(name="persist", bufs=1) as persist,
            tc.tile_pool(name="dram", bufs=1, space="DRAM") as dpool,
        ):
            h = dpool.tile([NP_TOTAL, D_OUT], bf16)

            w_sb = persist.tile([128, D_OUT], bf16)
            nc.sync.dma_start(w_sb[:], w_d)
            wsexp_sb = persist.tile([128, SLOTS, D_OUT], bf16)
            nc.sync.dma_start(wsexp_sb[:], wsexp_d.rearrange(
                "p (s c) -> p s c", s=SLOTS))
            wk1r_sb = persist.tile([128, SLOTS], f32)
            nc.sync.dma_start(wk1r_sb[:], wk1r_d)
            ecb_sb = persist.tile([128, D_OUT + 64], bf16)
            nc.sync.dma_start(ecb_sb[:], ecb_d)
            fbb_sb = persist.tile([128, D_OUT], bf16)
            nc.sync.dma_start(fbb_sb[:], fbb_d)
            ident_sb = persist.tile([128, 128], bf16)
            nc.sync.dma_start(ident_sb[:], ident_d)
            idx_sb = persist.tile([128, TILES * SLOTS], i32)
            nc.sync.dma_start(idx_sb[:], widx_d)

            # ---------- phase 1: h = x @ W (full, replicated) ----------
            with (
                tc.tile_pool(name="xsl", bufs=3) as xsl_p,
                tc.tile_pool(name="hsb", bufs=3) as hsb_p,
                tc.tile_pool(name="ps1", bufs=8, space="PSUM") as psum_p,
            ):
                for s in range(NSLABS):
                    xs = xsl_p.tile([128, SLAB], bf16, tag="xs")
                    nc.gpsimd.dma_start(xs[:], xt_d[:, s * SLAB:(s + 1) * SLAB])
                    hs = hsb_p.tile([128, SLAB // 128, D_OUT], bf16, tag="hs")
                    for j in range(0, SLAB // 128, 2):
                        pt = psum_p.tile([128, 512], f32, tag="pt")
                        nc.tensor.matmul(pt[:, 0:D_OUT],
                                         lhsT=xs[:, j * 128:(j + 1) * 128],
                                         rhs=w_sb[:], start=True, stop=True)
                        nc.tensor.matmul(pt[:, D_OUT:2 * D_OUT],
                                         lhsT=xs[:, (j + 1) * 128:(j + 2) * 128],
                                         rhs=w_sb[:], start=True, stop=True)
                        if j % 4 == 0:
                            nc.vector.tensor_copy(
                                hs[:, j:j + 2, :].rearrange("p j c -> p (j c)"),
                                pt[:])
                        else:
                            nc.scalar.activation(
                                hs[:, j:j + 2, :].rearrange("p j c -> p (j c)"),
                                pt[:], AF.Copy)
                    nc.sync.dma_start(
                        h[s * SLAB:(s + 1) * SLAB, :].rearrange(
                            "(j p) c -> p j c", p=128),
                        hs[:])

            # ---------- phase 2: gather + conv/softmax/pool ----------
            with (
                tc.tile_pool(name="reg", bufs=5) as reg_p,
                tc.tile_pool(name="work", bufs=3) as work,
                tc.tile_pool(name="ps2", bufs=3, space="PSUM") as ps2,
            ):
                PF = 3  # gather prefetch depth

                regions = {}

                def issue_gather(t):
                    r = reg_p.tile([128, SLOTS, D_OUT], bf16, tag="region",
                                   name=f"region{t}")
                    if BATCHED_GATHER:
                        idxs = idx_sb[:, t * SLOTS:(t + 1) * SLOTS]
                        nc.gpsimd.indirect_dma_start(
                            out=r[:], out_offset=None, in_=h[:, :],
                            in_offset=IndirectOffsetOnAxis(ap=idxs, axis=0))
                    else:
                        for sl in range(SLOTS):
                            nc.gpsimd.indirect_dma_start(
                                out=r[:, sl, :], out_offset=None, in_=h[:, :],
                                in_offset=IndirectOffsetOnAxis(
                                    ap=idx_sb[:, t * SLOTS + sl:t * SLOTS + sl + 1],
                                    axis=0))
                    regions[t] = r

                for t in range(PF):
                    issue_gather(t)
                for t in range(TILES):
                    if t + PF < TILES:
                        issue_gather(t + PF)
                    region = regions.pop(t)

                    # --- region * W: s-half on DVE, k-half on GpSimd ---
                    scal = work.tile([128, SLOTS, D_OUT], bf16, tag="scal")
                    if POOL_MUL:
                        nc.vector.tensor_mul(scal[:, 0:18, :], region[:, 0:18, :],
                                             wsexp_sb[:, 0:18, :])
                        nc.gpsimd.tensor_mul(scal[:, 18:SLOTS, :],
                                             region[:, 18:SLOTS, :],
                                             wsexp_sb[:, 18:SLOTS, :])
                    else:
                        nc.vector.tensor_mul(scal[:], region[:], wsexp_sb[:])

                    # --- conv slot-sums on PE (identity lhsT, PSUM acc) ---
                    ps_s = ps2.tile([128, 512], f32, tag="ps_s")
                    for u in range(KS):
                        nc.tensor.matmul(ps_s[:, 0:D_OUT], lhsT=ident_sb[:],
                                         rhs=scal[:, u, :],
                                         start=(u == 0), stop=(u == KS - 1))
                    ps_k = ps2.tile([128, 512], f32, tag="ps_k")
                    for u in range(KK):
                        nc.tensor.matmul(ps_k[:, 0:D_OUT], lhsT=ident_sb[:],
                                         rhs=scal[:, KS + u, :],
                                         start=(u == 0), stop=(u == KK - 1))

                    # --- softmax numerators: exp(conv)*exp(bias) ---
                    # k grouped conv first: sum channel groups of 4
                    ck = work.tile([128, 64], f32, tag="ck")
                    nc.vector.tensor_reduce(
                        ck[:], ps_k[:, 0:D_OUT].rearrange("p (o i) -> p o i", i=4),
                        axis=AX.X, op=ALU.add)
                    eall = work.tile([128, D_OUT + 64], bf16, tag="eall")
                    nc.scalar.activation(eall[:, 0:D_OUT], ps_s[:, 0:D_OUT],
                                         AF.Exp)
                    nc.scalar.activation(eall[:, D_OUT:D_OUT + 64], ck[:], AF.Exp)
                    eallE = work.tile([128, D_OUT + 64], bf16, tag="eallE")
                    nc.vector.tensor_mul(eallE[:], eall[:], ecb_sb[:])
                    esE = eallE[:, 0:D_OUT].rearrange("p (i j) -> p i j", j=KS)
                    ekE = eallE[:, D_OUT:D_OUT + 64].rearrange(
                        "p (i j) -> p i j", j=KK)
                    sume = work.tile([128, KS], f32, tag="sume")
                    nc.vector.tensor_reduce(sume[:], esE, axis=AX.X, op=ALU.add)
                    rec = work.tile([128, KS], f32, tag="rec")
                    nc.vector.reciprocal(rec[:], sume[:])
                    r2 = work.tile([128, KS], f32, tag="r2")
                    nc.vector.tensor_mul(r2[:], rec[:], wk1r_sb[:, 0:KS])
                    ps_ = work.tile([128, KS, KS], bf16, tag="ps_")
                    nc.vector.tensor_mul(ps_[:], esE,
                                         r2.to_broadcast([128, KS, KS]))
                    beta = work.tile([128, SLOTS], f32, tag="beta")
                    nc.vector.tensor_reduce(beta[:, 0:KS],
                                            ps_.rearrange("p i j -> p j i"),
                                            axis=AX.X, op=ALU.add)

                    sumk = work.tile([128, KK], f32, tag="sumk")
                    nc.vector.tensor_reduce(sumk[:], ekE, axis=AX.X, op=ALU.add)
                    reck = work.tile([128, KK], f32, tag="reck")
                    nc.vector.reciprocal(reck[:], sumk[:])
                    r2k = work.tile([128, KK], f32, tag="r2k")
                    nc.vector.tensor_mul(r2k[:], reck[:], wk1r_sb[:, KS:SLOTS])
                    pk_ = work.tile([128, KK, KK], bf16, tag="pk_")
                    nc.vector.tensor_mul(pk_[:], ekE,
                                         r2k.to_broadcast([128, KK, KK]))
                    nc.vector.tensor_reduce(beta[:, KS:SLOTS],
                                            pk_.rearrange("p i j -> p j i"),
                                            axis=AX.X, op=ALU.add)

                    # --- pooled on PE: sum_j diag(beta_j) @ region_j + bias ---
                    diag = work.tile([128, SLOTS, 128], bf16, tag="diag")
                    for j in range(SLOTS):
                        if ACT_DIAG and j % 3 != 0:
                            nc.scalar.activation(diag[:, j, :], ident_sb[:],
                                                 AF.Copy, scale=beta[:, j:j + 1])
                        else:
                            nc.vector.tensor_scalar_mul(diag[:, j, :], ident_sb[:],
                                                        beta[:, j:j + 1])
                    ps_o = ps_s  # reuse the s-conv PSUM bank
                    for j in range(SLOTS):
                        nc.tensor.matmul(ps_o[:, 0:D_OUT], lhsT=diag[:, j, :],
                                         rhs=region[:, j, :],
                                         start=(j == 0), stop=False)
                    nc.tensor.matmul(ps_o[:, 0:D_OUT], lhsT=ident_sb[:],
                                     rhs=fbb_sb[:], start=False, stop=True)

                    outs = work.tile([128, D_OUT], bf16, tag="outs")
                    nc.scalar.activation(outs[:], ps_o[:, 0:D_OUT], AF.Copy)
                    nc.sync.dma_start(out_d[t * 128:(t + 1) * 128, :], outs[:])

    nc.finalize()
    return nc


def _prep_inputs(inputs):
    import ml_dtypes
    bf16 = ml_dtypes.bfloat16

    x = np.asarray(inputs["x"], dtype=np.float32)
    edge = np.asarray(inputs["edge_neighs_index"], dtype=np.int32)
    knn = np.asarray(inputs["knn_neighs_index"], dtype=np.int32)
    W = np.asarray(inputs["weight"], dtype=np.float32)
    bias = np.asarray(inputs["bias"], dtype=np.float32)
    ws = np.asarray(inputs["convKK_s_w"], dtype=np.float32)     # (256,1,16)
    wsb = np.asarray(inputs["convKK_s_b"], dtype=np.float32)    # (256,)
    ws1 = np.asarray(inputs["convK1_s_w"], dtype=np.float32)    # (1,16,1)
    ws1b = np.asarray(inputs["convK1_s_b"], dtype=np.float32)   # (1,)
    wk = np.asarray(inputs["convKK_k_w"], dtype=np.float32)     # (64,4,8)
    wkb = np.asarray(inputs["convKK_k_b"], dtype=np.float32)    # (64,)
    wk1 = np.asarray(inputs["convK1_k_w"], dtype=np.float32)    # (1,8,1)
    wk1b = np.asarray(inputs["convK1_k_b"], dtype=np.float32)   # (1,)

    xp = np.zeros((NP_TOTAL, D_IN), np.float32)
    xp[:N] = x
    xT = np.ascontiguousarray(xp.T).astype(bf16)                 # (128, 50176)
    Wb = W.astype(bf16)                                          # (128, 256)

    merged = np.zeros((NP_TOTAL, SLOTS), np.int32)
    merged[:N, :KS] = edge
    merged[:N, KS:] = knn

    # WsE[t, c] = ws[c, 0, t];  WkE[t, o*4+i] = wk[o, i, t]
    WsE = ws[:, 0, :].T                                          # (16, 256)
    WkE = wk.transpose(2, 0, 1).reshape(KK, 256)                 # (8, 256)
    wsexp = np.concatenate([WsE.reshape(-1), WkE.reshape(-1)])
    wsexp_t = np.ascontiguousarray(
        np.broadcast_to(wsexp, (128, SLOTS * D_OUT))).astype(bf16)

    wk1r = np.ascontiguousarray(np.broadcast_to(
        np.concatenate([ws1[0, :, 0], wk1[0, :, 0]]), (128, SLOTS))
    ).astype(np.float32)
    # exp of the conv biases, premultiplied into the softmax numerator
    ecb = np.ascontiguousarray(np.broadcast_to(
        np.exp(np.concatenate([wsb, wkb])), (128, D_OUT + 64))).astype(bf16)
    # final bias row (replicated across partitions) added via matmul
    fbb = np.ascontiguousarray(np.broadcast_to(
        bias + ws1b[0] + wk1b[0], (128, D_OUT))).astype(bf16)
    ident = np.eye(128, dtype=np.float32).astype(bf16)

    in_maps = []
    for c in range(NCORES):
        widx_c = np.ascontiguousarray(
            merged[c * PER_CORE:(c + 1) * PER_CORE]
            .reshape(TILES, 128, SLOTS).transpose(1, 0, 2)
            .reshape(128, TILES * SLOTS))
        in_maps.append({
            "xt": xT, "wmat": Wb, "widx": widx_c, "wsexp": wsexp_t,
            "wk1r": wk1r, "ecb": ecb, "fbb": fbb, "ident": ident,
        })
    return in_maps


_CACHED_NC = None


def run(inputs, trace=False):
    """Build (cached), run on 8 cores, return (output, BassKernelResults)."""
    global _CACHED_NC
    from concourse.bass_utils import run_bass_kernel_spmd

    if _CACHED_NC is None:
        _CACHED_NC = _build_program()
    nc = _CACHED_NC

    in_maps = _prep_inputs(inputs)
    res = run_bass_kernel_spmd(nc, in_maps, core_ids=list(range(NCORES)),
                               trace=trace)
    shards = [np.asarray(res.results[c]["out"], dtype=np.float32)
              for c in range(NCORES)]
    full = np.concatenate(shards, axis=0)[:N]
    return full, res


def kernel(**inputs) -> np.ndarray:
    out, _ = run(inputs, trace=False)
    return out
